# revision 11
# baseline (speedup 1.0000x reference)
"""DiffTreeInterpreter scatter-coalesce kernel for 8 Trainium2 cores.

Data-parallel over batch B=32: core c owns batches [4c, 4c+4). Host work
is index prep only: bucketing entries by (batch, role-group), shipping
bit-exact copies of per-entry weights; all arithmetic happens on device.

Math (see reference): with H = R/2, entry n (b, l, r, v=mem[n],
w=arg_weights[b,l]) contributes to out[b] at up to 3 bins:
  bin r>>1  with uA = op0*w0 (r even) / op1*w1 (r odd, r!=1)   [A-stream]
  bin 2r    with u2 = op2*w2  (only r < H)                     [cons even]
  bin 2r+1  with u3 = op2*w3  (only r < H)                     [cons odd]
plus out[b,1] += op2*root_filler[b]. (Pad-mask is a no-op: masked rows
are all-zero values.)

Device design (per batch):
- 16 groups of 256 roles, 5 value tiles (128 entries) each = 80 tiles.
  Lower groups g<8 chain-pack their 4 cons blocks (64 roles) into the 5
  tiles with static (tile, block) incidence T0:A T1:AB T2:BC T3:CD T4:D.
- One-hot slabs [entry-partition, cols] fp16: lower = merged [640 A-cols
  | 1024 cons-cols] built by one GPSIMD local_scatter; upper A-slabs
  [640] built per-tile on DVE (iota EQ r1 * u) or ACT (2-op tent:
  Square then Relu with per-partition bias/scale).
- Matmuls are value-stationary: out[f, bins] = v[entry,F]^T @ slab.
  PSUM holds 8 "superblock" banks [128, 512 bins] per batch; A-stream
  and cons matmuls accumulate into shared bank slices (A starts a
  slice, cons of the owning lower group finishes it), so no separate
  combine pass exists.
- Banks drain via one wide ACT copy [128,512] into paired staging, then
  DMA to out[b, F, R] (host transposes at unshard).
"""

import sys

if "/opt/trn_rl_repo" not in sys.path:
    sys.path.insert(0, "/opt/trn_rl_repo")

import numpy as np

B, L, F, R = 32, 128, 128, 4096
H = R >> 1
N = 262144
NCORES = 8
BPC = B // NCORES
P = 128

NG = 16          # role groups per batch (256 roles each)
TPG = 5          # value tiles per group
TILES_PER_BATCH = NG * TPG  # 80
NT = BPC * TILES_PER_BATCH  # 320 tiles per core
VB = 16          # value tiles per load DMA
NVS = NT // VB   # 20 value load slabs per core

SLAB_A = TPG * P        # 640
SLAB_W = SLAB_A + 8 * P  # 1664 (A cols + 8 cons ranges)

# upper-group one-hot builder assignment (groups 8..15)
DVE_G = (8, 9, 10, 11, 12, 13)
ACT_G = (14, 15)

# per-batch step schedule: upper groups first (their slabs build on
# DVE/ACT while the GPSIMD library loads), cons/drains spread across the
# batch. cons(k) only needs its bank's start=True matmul done (banks
# 0-3: the A of group 4k; banks 4-7: cons itself starts the bank).
ORDER_A = (8, 9, 10, 11, 12, 13, 14, 15, 0, 1, 2, 3, 4, 5, 6, 7)
CONS_AT = {1: 2, 5: 3, 8: 4, 9: 5, 10: 6, 11: 7, 12: 0, 13: 1}
DRAIN_AT = {3: 2, 7: 3, 8: 4, 9: 5, 10: 6, 11: 7, 12: 0, 15: 1}
# value stream group order = first-use order of each group's tiles
STREAM_G = (8, 9, 2, 10, 11, 12, 13, 3, 14, 15, 0, 4, 1, 5, 6, 7)
SPOS = {g: i for i, g in enumerate(STREAM_G)}

_PROG_CACHE = {}


def _build_program():
    import concourse.bacc as bacc
    import concourse.mybir as mybir
    import concourse.tile as tile

    fp32 = mybir.dt.float32
    fp16 = mybir.dt.float16
    i16 = mybir.dt.int16
    MUL = mybir.AluOpType.mult
    EQ = mybir.AluOpType.is_equal
    AF = mybir.ActivationFunctionType

    nc = bacc.Bacc(None, target_bir_lowering=False)
    vals = nc.dram_tensor("vals", [NVS, P, VB, F], fp16, kind="ExternalInput")
    # fp16 meta: [0:160) = (WA, OPA) per slot (g*5+t); [160:280) = lower
    # (WB, WC, OP2) per (g, t)
    meta = nc.dram_tensor("meta", [BPC, P, 280], fp16, kind="ExternalInput")
    # fp32 (r1, -r1) per upper slot ((g-8)*5+t)
    r1pm = nc.dram_tensor("r1pm", [BPC, P, 40, 2], fp32, kind="ExternalInput")
    idxs = nc.dram_tensor("idxs", [BPC, P, NG, 16], i16, kind="ExternalInput")
    iota = nc.dram_tensor("iota", [P, P], fp16, kind="ExternalInput")
    out = nc.dram_tensor("out", [BPC, F, R], fp16, kind="ExternalOutput")

    with tile.TileContext(nc) as tc:
        with tc.tile_pool(name="cst", bufs=1) as cpool, \
             tc.tile_pool(name="meta", bufs=BPC) as mpool, \
             tc.tile_pool(name="ud", bufs=BPC) as upool, \
             tc.tile_pool(name="vload", bufs=10) as vpool, \
             tc.tile_pool(name="slab", bufs=13) as spool, \
             tc.tile_pool(name="sq", bufs=2) as qpool, \
             tc.tile_pool(name="stage", bufs=8) as gpool, \
             tc.tile_pool(name="bank", bufs=8, space="PSUM") as bpool:

            io_t = cpool.tile([P, P], fp16, tag="iota")
            nc.sync.dma_start(out=io_t[:], in_=iota[:])

            vtiles = {}

            def vload(vs):
                if vs not in vtiles:
                    vt = vpool.tile([P, VB, F], fp16, tag="v")
                    nc.sync.dma_start(out=vt[:], in_=vals[vs])
                    vtiles[vs] = vt

            # prefetch metadata for all batches + compute u products
            metas = []
            for b in range(BPC):
                m = mpool.tile([P, 280], fp16, tag="m")
                nc.sync.dma_start(out=m[:], in_=meta[b])
                rp = mpool.tile([P, 40, 2], fp32, tag="rp")
                nc.sync.dma_start(out=rp[:], in_=r1pm[b])
                x = mpool.tile([P, NG, 16], i16, tag="x")
                nc.sync.dma_start(out=x[:], in_=idxs[b])

                m1 = m[:, 0:160].rearrange("p (s c) -> p s c", c=2)
                m2 = m[:, 160:280].rearrange("p (g t c) -> p g t c", g=8, c=3)
                ud = upool.tile([P, NG, 16], fp16, tag="ud")
                # uA = WA*OPA into ud[:, :, 0:5]
                nc.vector.tensor_tensor(
                    out=ud[:, :, 0:5],
                    in0=m1[:, :, 0].rearrange("p (g t) -> p g t", t=TPG),
                    in1=m1[:, :, 1].rearrange("p (g t) -> p g t", t=TPG),
                    op=MUL)
                # u2 = WB*OP2 into ud[:, 0:8, 5:10]; u3 = WC*OP2 into [10:15]
                nc.vector.tensor_tensor(
                    out=ud[:, 0:8, 5:10], in0=m2[:, :, :, 0],
                    in1=m2[:, :, :, 2], op=MUL)
                nc.vector.tensor_tensor(
                    out=ud[:, 0:8, 10:15], in0=m2[:, :, :, 1],
                    in1=m2[:, :, :, 2], op=MUL)
                # fp32 u (and -u) for upper-slot scalar operands
                u1f = upool.tile([P, 40], fp32, tag="u1f")
                nc.vector.tensor_tensor(
                    out=u1f[:], in0=m1[:, 40:80, 0], in1=m1[:, 40:80, 1],
                    op=MUL)
                ngu = upool.tile([P, 40], fp32, tag="ngu")
                nc.vector.tensor_scalar(
                    out=ngu[:], in0=u1f[:], scalar1=-1.0, scalar2=None,
                    op0=MUL)
                metas.append((ud, x, u1f, ngu, rp))
                if b == 0:
                    vload(0)
                    vload(1)

            for b in range(BPC):
                ud, x, u1f, ngu, rp = metas[b]

                def vtile(g, tl):
                    t = b * TILES_PER_BATCH + SPOS[g] * TPG + tl
                    vload(t // VB)
                    if (t // VB) + 1 < NVS:
                        vload(t // VB + 1)
                    return vtiles[t // VB][:, t % VB, :]

                banks = [None] * 8
                slabs = [None] * NG
                stages = [None] * 4

                def drain(k):
                    # pair (k, k^1) shares a staging tile; DMA on 2nd drain
                    pair = k >> 1
                    if stages[pair] is None:
                        stages[pair] = gpool.tile([P, 1024], fp16, tag="st", name="st")
                    st = stages[pair]
                    half = (k & 1) * 512
                    nc.scalar.copy(out=st[:, half:half + 512],
                                   in_=banks[k][:])
                    if k & 1:
                        nc.sync.dma_start(
                            out=out[b, :, (pair * 1024):(pair * 1024 + 1024)],
                            in_=st[:])

                def build_slab(g):
                    if slabs[g] is not None:
                        return slabs[g]
                    sl = spool.tile([P, SLAB_W], fp16, tag="sl", name="sl")
                    slabs[g] = sl
                    if g < 8:
                        nc.gpsimd.local_scatter(
                            out_ap=sl[:], data_ap=ud[:, g, :],
                            idxs_ap=x[:, g, :],
                            channels=P, num_elems=SLAB_W, num_idxs=16)
                    elif g in DVE_G:
                        for tl in range(TPG):
                            s = (g - 8) * TPG + tl
                            nc.vector.tensor_scalar(
                                out=sl[:, tl * P:(tl + 1) * P], in0=io_t[:],
                                scalar1=rp[:, s, 0:1],
                                scalar2=u1f[:, s:s + 1],
                                op0=EQ, op1=MUL)
                    else:
                        for tl in range(TPG):
                            s = (g - 8) * TPG + tl
                            sq = qpool.tile([P, P], fp16, tag="sq", name="sq")
                            nc.scalar.activation(
                                out=sq[:], in_=io_t[:], func=AF.Square,
                                bias=rp[:, s, 1:2], scale=1.0)
                            nc.scalar.activation(
                                out=sl[:, tl * P:(tl + 1) * P], in_=sq[:],
                                func=AF.Relu, bias=u1f[:, s:s + 1],
                                scale=ngu[:, s:s + 1])
                    return sl

                def cons(cg, bank_fresh):
                    # group cg's cons into bank cg; block j gets
                    # (T_j: range 2j, T_{j+1}: range 2j+1). PSUM
                    # start=True clears has_written for the WHOLE bank,
                    # so only a bank's first-ever matmul may set it;
                    # start=False overwrites where the bit is clear and
                    # accumulates where set.
                    csl = build_slab(cg)
                    for tl in range(TPG):
                        v = vtile(cg, tl)
                        if tl >= 1:  # tile ends block tl-1
                            rng = SLAB_A + (2 * (tl - 1) + 1) * P
                            nc.tensor.matmul(
                                out=banks[cg][:, (tl - 1) * P:tl * P],
                                lhsT=v, rhs=csl[:, rng:rng + P],
                                start=False, stop=True,
                                skip_group_check=True)
                        if tl <= 3:  # tile starts block tl
                            rng = SLAB_A + (2 * tl) * P
                            nc.tensor.matmul(
                                out=banks[cg][:, tl * P:(tl + 1) * P],
                                lhsT=v, rhs=csl[:, rng:rng + P],
                                start=(bank_fresh and tl == 0),
                                stop=False, skip_group_check=True)

                for step, g in enumerate(ORDER_A):
                    sl = build_slab(g)

                    # --- A-stream matmuls: bank g>>2, slice g&3 ---
                    bk = g >> 2
                    first = banks[bk] is None
                    if first:
                        banks[bk] = bpool.tile([P, 512], fp32, tag="bk", name="bk")
                    sli = (g & 3) * P
                    for tl in range(TPG):
                        v = vtile(g, tl)
                        nc.tensor.matmul(
                            out=banks[bk][:, sli:sli + P], lhsT=v,
                            rhs=sl[:, tl * P:(tl + 1) * P],
                            start=(first and tl == 0), stop=False,
                            skip_group_check=True)

                    # --- cons + drain per schedule ---
                    if step in CONS_AT:
                        cg = CONS_AT[step]
                        fresh = banks[cg] is None
                        if fresh:
                            banks[cg] = bpool.tile([P, 512], fp32, tag="bk", name="bk")
                        cons(cg, bank_fresh=fresh)
                    if step in DRAIN_AT:
                        drain(DRAIN_AT[step])

    nc.compile()
    return nc


def _pack_inputs(mem_values, arg_weights, root_filler, op_dist,
                 batch_idx, slot_idx, role_idx):
    """Host-side sharding/packing: index selection and copies only."""
    mem_values = np.ascontiguousarray(mem_values, dtype=np.float32)
    arg_weights = np.asarray(arg_weights, dtype=np.float32)
    root_filler = np.asarray(root_filler, dtype=np.float32)
    op_dist = np.asarray(op_dist, dtype=np.float32)
    batch_idx = np.asarray(batch_idx, dtype=np.int64)
    slot_idx = np.asarray(slot_idx, dtype=np.int64)
    role_idx = np.asarray(role_idx, dtype=np.int64)

    w = arg_weights[batch_idx, slot_idx]  # [N, 4] gathered copies
    r = role_idx
    even = (r & 1) == 0
    wA = np.where(even, w[:, 0], np.where(r != 1, w[:, 1], 0.0))
    opA = np.where(even, op_dist[batch_idx, 0], op_dist[batch_idx, 1])

    iota_np = np.broadcast_to(
        np.arange(P, dtype=np.float16), (P, P)).copy()

    in_maps = []
    for c in range(NCORES):
        vals_s = np.zeros((NT * P, F), np.float16)
        meta_s = np.zeros((BPC, P, 280), np.float16)
        r1pm_s = np.zeros((BPC, P, 40, 2), np.float32)
        r1pm_s[:, :, :, 0] = -1.0
        r1pm_s[:, :, :, 1] = 1.0
        idx_s = np.full((BPC, P, NG, 16), -1, np.int16)

        for bb in range(BPC):
            b = c * BPC + bb
            sel0 = np.nonzero(batch_idx == b)[0]
            rr0 = r[sel0]
            for g in range(NG):
                gsel = sel0[(rr0 >> 8) == g]
                rg = r[gsel]
                if g < 8:
                    j = (rg >> 6) & 3
                    order = np.argsort(j, kind="stable")
                    gsel, rg, j = gsel[order], rg[order], j[order]
                    cnt = np.bincount(j, minlength=4)
                    is_root = np.zeros(rg.size, bool)
                    if g == 0:
                        # synthetic root entry joins block 0's stream end
                        ins = cnt[0]
                        gsel = np.insert(gsel, ins, -1)
                        rg = np.insert(rg, ins, 0)
                        j = np.insert(j, ins, 0)
                        is_root = np.insert(is_root, ins, True)
                        cnt[0] += 1
                    start = np.zeros(4, np.int64)
                    pos_in = np.arange(rg.size) - np.concatenate(
                        [[0], np.cumsum(cnt)])[:-1][j]
                    e = 0
                    for blk in range(4):
                        start[blk] = max(e, 128 * blk)
                        e = start[blk] + cnt[blk]
                    if cnt.max() > 256 or e > SLAB_A or \
                       (start[:3] + cnt[:3] > [256, 384, 512]).any():
                        raise RuntimeError("chain capacity exceeded")
                    pos = start[j] + pos_in
                    tl = pos >> 7
                    if (tl > j + 1).any() or (tl < j).any():
                        raise RuntimeError("chain incidence violated")
                    rng = np.where(tl == j, 2 * j, 2 * j + 1)
                    c2 = SLAB_A + rng * P + 2 * (rg & 63)
                else:
                    order = np.argsort(rg, kind="stable")
                    gsel, rg = gsel[order], rg[order]
                    if rg.size > SLAB_A:
                        raise RuntimeError("upper capacity exceeded")
                    pos = np.arange(rg.size)
                    tl = pos >> 7
                    is_root = np.zeros(rg.size, bool)

                p = pos & 127
                r1 = (rg >> 1) & 127
                acol = tl * P + r1
                slot = g * TPG + tl
                t_global = bb * TILES_PER_BATCH + SPOS[g] * TPG + tl

                real = ~is_root
                vals_s[t_global * P + p] = np.where(
                    is_root[:, None], root_filler[b].astype(np.float16),
                    mem_values[gsel].astype(np.float16))
                # meta1: (WA, OPA) at [slot*2], zeros for root
                meta_s[bb, p[real], slot[real] * 2] = wA[gsel[real]]
                meta_s[bb, p[real], slot[real] * 2 + 1] = opA[gsel[real]]
                # A one-hot idx (col 0..4 by tile) — skip root
                idx_s[bb, p[real], g, tl[real]] = acol[real]
                if g < 8:
                    # meta2: (WB, WC, OP2) at [160 + (g*5+tl)*3]
                    base = 160 + slot * 3
                    meta_s[bb, p[real], base[real]] = w[gsel[real], 2]
                    meta_s[bb, p[real], base[real] + 1] = w[gsel[real], 3]
                    meta_s[bb, p, base + 2] = op_dist[b, 2]
                    if is_root.any():
                        meta_s[bb, p[is_root], base[is_root] + 1] = 1.0
                    idx_s[bb, p[real], g, 5 + tl[real]] = c2[real]
                    idx_s[bb, p, g, 10 + tl] = c2 + 1
                    if is_root.any():
                        # root has no even-bin write
                        idx_s[bb, p[is_root], g, 5 + tl[is_root]] = -1
                else:
                    us = (g - 8) * TPG + tl
                    r1pm_s[bb, p, us, 0] = r1
                    r1pm_s[bb, p, us, 1] = -r1.astype(np.float32)

        in_maps.append({
            "vals": np.ascontiguousarray(
                vals_s.reshape(NVS, VB, P, F).transpose(0, 2, 1, 3)),
            "meta": meta_s,
            "r1pm": r1pm_s,
            "idxs": idx_s,
            "iota": iota_np,
        })
    return in_maps


def kernel(**inputs):
    from concourse.bass_utils import run_bass_kernel_spmd

    in_maps = _pack_inputs(**inputs)
    if "nc" not in _PROG_CACHE:
        _PROG_CACHE["nc"] = _build_program()
    nc = _PROG_CACHE["nc"]
    res = run_bass_kernel_spmd(nc, in_maps, list(range(NCORES)))
    return np.ascontiguousarray(np.concatenate(
        [res.results[c]["out"].transpose(0, 2, 1) for c in range(NCORES)],
        axis=0).astype(np.float32))


# revision 12
# speedup vs baseline: 1.0111x; 1.0111x over previous
"""DiffTreeInterpreter scatter-coalesce kernel for 8 Trainium2 cores.

Data-parallel over batch B=32: core c owns batches [4c, 4c+4). Host work
is index prep only: bucketing entries by (batch, role-group), shipping
bit-exact copies of per-entry weights; all arithmetic happens on device.

Math (see reference): with H = R/2, entry n (b, l, r, v=mem[n],
w=arg_weights[b,l]) contributes to out[b] at up to 3 bins:
  bin r>>1  with uA = op0*w0 (r even) / op1*w1 (r odd, r!=1)   [A-stream]
  bin 2r    with u2 = op2*w2  (only r < H)                     [cons even]
  bin 2r+1  with u3 = op2*w3  (only r < H)                     [cons odd]
plus out[b,1] += op2*root_filler[b]. (Pad-mask is a no-op: masked rows
are all-zero values.)

Device design (per batch):
- 16 groups of 256 roles, 5 value tiles (128 entries) each = 80 tiles.
  Lower groups g<8 chain-pack their 4 cons blocks (64 roles) into the 5
  tiles with static (tile, block) incidence T0:A T1:AB T2:BC T3:CD T4:D.
- One-hot slabs [entry-partition, cols] fp16: lower = merged [640 A-cols
  | 1024 cons-cols] built by one GPSIMD local_scatter; upper A-slabs
  [640] built per-tile on DVE (iota EQ r1 * u) or ACT (2-op tent:
  Square then Relu with per-partition bias/scale).
- Matmuls are value-stationary: out[f, bins] = v[entry,F]^T @ slab.
  PSUM holds 8 "superblock" banks [128, 512 bins] per batch; A-stream
  and cons matmuls accumulate into shared bank slices (A starts a
  slice, cons of the owning lower group finishes it), so no separate
  combine pass exists.
- Banks drain via one wide ACT copy [128,512] into paired staging, then
  DMA to out[b, F, R] (host transposes at unshard).
"""

import sys

if "/opt/trn_rl_repo" not in sys.path:
    sys.path.insert(0, "/opt/trn_rl_repo")

import numpy as np

B, L, F, R = 32, 128, 128, 4096
H = R >> 1
N = 262144
NCORES = 8
BPC = B // NCORES
P = 128

NG = 16          # role groups per batch (256 roles each)
TPG = 5          # value tiles per group
TILES_PER_BATCH = NG * TPG  # 80
NT = BPC * TILES_PER_BATCH  # 320 tiles per core
VB = 16          # value tiles per load DMA
NVS = NT // VB   # 20 value load slabs per core

SLAB_A = TPG * P        # 640
SLAB_W = SLAB_A + 8 * P  # 1664 (A cols + 8 cons ranges)

# upper-group one-hot builder assignment (groups 8..15)
DVE_G = (8, 9, 10, 11, 12, 13)
ACT_G = (14, 15)

# per-batch step schedule: upper groups first (their slabs build on
# DVE/ACT while the GPSIMD library loads), cons/drains spread across the
# batch. cons(k) only needs its bank's start=True matmul done (banks
# 0-3: the A of group 4k; banks 4-7: cons itself starts the bank).
ORDER_A = (8, 9, 10, 11, 12, 13, 14, 15, 0, 1, 2, 3, 4, 5, 6, 7)
CONS_AT = {3: 2, 7: 3, 8: 4, 9: 5, 10: 6, 11: 7, 12: 0, 15: 1}
DRAIN_AT = {3: 2, 7: 3, 8: 4, 9: 5, 10: 6, 11: 7, 12: 0, 15: 1}
# banks drained on the DVE engine (its builds end by mid-batch)
DVE_DRAIN = (4, 5, 6, 7)
# value stream group order = first-use order of each group's tiles
STREAM_G = (8, 9, 10, 11, 12, 13, 14, 15, 0, 4, 1, 5, 2, 6, 3, 7)
SPOS = {g: i for i, g in enumerate(STREAM_G)}

_PROG_CACHE = {}


def _build_program():
    import concourse.bacc as bacc
    import concourse.mybir as mybir
    import concourse.tile as tile

    fp32 = mybir.dt.float32
    fp16 = mybir.dt.float16
    i16 = mybir.dt.int16
    MUL = mybir.AluOpType.mult
    EQ = mybir.AluOpType.is_equal
    AF = mybir.ActivationFunctionType

    nc = bacc.Bacc(None, target_bir_lowering=False)
    vals = nc.dram_tensor("vals", [NVS, P, VB, F], fp16, kind="ExternalInput")
    # fp16 meta: [0:160) = (WA, OPA) per slot (g*5+t); [160:280) = lower
    # (WB, WC, OP2) per (g, t)
    meta = nc.dram_tensor("meta", [BPC, P, 280], fp16, kind="ExternalInput")
    # fp32 (r1, -r1) per upper slot ((g-8)*5+t)
    r1pm = nc.dram_tensor("r1pm", [BPC, P, 40, 2], fp32, kind="ExternalInput")
    idxs = nc.dram_tensor("idxs", [BPC, P, NG, 16], i16, kind="ExternalInput")
    iota = nc.dram_tensor("iota", [P, P], fp16, kind="ExternalInput")
    out = nc.dram_tensor("out", [BPC, F, R], fp16, kind="ExternalOutput")

    with tile.TileContext(nc) as tc:
        with tc.tile_pool(name="cst", bufs=1) as cpool, \
             tc.tile_pool(name="meta", bufs=BPC) as mpool, \
             tc.tile_pool(name="ud", bufs=BPC) as upool, \
             tc.tile_pool(name="vload", bufs=10) as vpool, \
             tc.tile_pool(name="slab", bufs=13) as spool, \
             tc.tile_pool(name="sq", bufs=2) as qpool, \
             tc.tile_pool(name="stage", bufs=8) as gpool, \
             tc.tile_pool(name="bank", bufs=8, space="PSUM") as bpool:

            io_t = cpool.tile([P, P], fp16, tag="iota")
            nc.sync.dma_start(out=io_t[:], in_=iota[:])

            vtiles = {}

            def vload(vs):
                if vs not in vtiles:
                    vt = vpool.tile([P, VB, F], fp16, tag="v")
                    nc.sync.dma_start(out=vt[:], in_=vals[vs])
                    vtiles[vs] = vt

            # prefetch metadata for all batches + compute u products
            metas = []
            for b in range(BPC):
                m = mpool.tile([P, 280], fp16, tag="m")
                nc.sync.dma_start(out=m[:], in_=meta[b])
                rp = mpool.tile([P, 40, 2], fp32, tag="rp")
                nc.sync.dma_start(out=rp[:], in_=r1pm[b])
                x = mpool.tile([P, NG, 16], i16, tag="x")
                nc.sync.dma_start(out=x[:], in_=idxs[b])

                m1 = m[:, 0:160].rearrange("p (s c) -> p s c", c=2)
                m2 = m[:, 160:280].rearrange("p (g t c) -> p g t c", g=8, c=3)
                ud = upool.tile([P, NG, 16], fp16, tag="ud")
                # uA = WA*OPA into ud[:, :, 0:5]
                nc.vector.tensor_tensor(
                    out=ud[:, :, 0:5],
                    in0=m1[:, :, 0].rearrange("p (g t) -> p g t", t=TPG),
                    in1=m1[:, :, 1].rearrange("p (g t) -> p g t", t=TPG),
                    op=MUL)
                # u2 = WB*OP2 into ud[:, 0:8, 5:10]; u3 = WC*OP2 into [10:15]
                nc.vector.tensor_tensor(
                    out=ud[:, 0:8, 5:10], in0=m2[:, :, :, 0],
                    in1=m2[:, :, :, 2], op=MUL)
                nc.vector.tensor_tensor(
                    out=ud[:, 0:8, 10:15], in0=m2[:, :, :, 1],
                    in1=m2[:, :, :, 2], op=MUL)
                # fp32 u (and -u) for upper-slot scalar operands
                u1f = upool.tile([P, 40], fp32, tag="u1f")
                nc.vector.tensor_tensor(
                    out=u1f[:], in0=m1[:, 40:80, 0], in1=m1[:, 40:80, 1],
                    op=MUL)
                ngu = upool.tile([P, 40], fp32, tag="ngu")
                nc.vector.tensor_scalar(
                    out=ngu[:], in0=u1f[:], scalar1=-1.0, scalar2=None,
                    op0=MUL)
                metas.append((ud, x, u1f, ngu, rp))
                if b == 0:
                    vload(0)
                    vload(1)

            for b in range(BPC):
                ud, x, u1f, ngu, rp = metas[b]

                def vtile(g, tl):
                    t = b * TILES_PER_BATCH + SPOS[g] * TPG + tl
                    vload(t // VB)
                    if (t // VB) + 1 < NVS:
                        vload(t // VB + 1)
                    return vtiles[t // VB][:, t % VB, :]

                banks = [None] * 8
                slabs = [None] * NG
                stages = [None] * 4

                def drain(k):
                    # pair (k, k^1) shares a staging tile; DMA on 2nd drain
                    pair = k >> 1
                    if stages[pair] is None:
                        stages[pair] = gpool.tile([P, 1024], fp16, tag="st", name="st")
                    st = stages[pair]
                    half = (k & 1) * 512
                    if k in DVE_DRAIN:
                        nc.vector.tensor_scalar(
                            out=st[:, half:half + 512], in0=banks[k][:],
                            scalar1=1.0, scalar2=None, op0=MUL)
                    else:
                        nc.scalar.copy(out=st[:, half:half + 512],
                                       in_=banks[k][:])
                    if k & 1:
                        nc.sync.dma_start(
                            out=out[b, :, (pair * 1024):(pair * 1024 + 1024)],
                            in_=st[:])

                def build_slab(g):
                    if slabs[g] is not None:
                        return slabs[g]
                    sl = spool.tile([P, SLAB_W], fp16, tag="sl", name="sl")
                    slabs[g] = sl
                    if g < 8:
                        nc.gpsimd.local_scatter(
                            out_ap=sl[:], data_ap=ud[:, g, :],
                            idxs_ap=x[:, g, :],
                            channels=P, num_elems=SLAB_W, num_idxs=16)
                    elif g in DVE_G:
                        for tl in range(TPG):
                            s = (g - 8) * TPG + tl
                            nc.vector.tensor_scalar(
                                out=sl[:, tl * P:(tl + 1) * P], in0=io_t[:],
                                scalar1=rp[:, s, 0:1],
                                scalar2=u1f[:, s:s + 1],
                                op0=EQ, op1=MUL)
                    else:
                        for tl in range(TPG):
                            s = (g - 8) * TPG + tl
                            sq = qpool.tile([P, P], fp16, tag="sq", name="sq")
                            nc.scalar.activation(
                                out=sq[:], in_=io_t[:], func=AF.Square,
                                bias=rp[:, s, 1:2], scale=1.0)
                            nc.scalar.activation(
                                out=sl[:, tl * P:(tl + 1) * P], in_=sq[:],
                                func=AF.Relu, bias=u1f[:, s:s + 1],
                                scale=ngu[:, s:s + 1])
                    return sl

                def cons(cg, bank_fresh):
                    # group cg's cons into bank cg; block j gets
                    # (T_j: range 2j, T_{j+1}: range 2j+1). PSUM
                    # start=True clears has_written for the WHOLE bank,
                    # so only a bank's first-ever matmul may set it;
                    # start=False overwrites where the bit is clear and
                    # accumulates where set.
                    csl = build_slab(cg)
                    for tl in range(TPG):
                        # tile tl covers ranges (2tl-1, 2tl) = contiguous
                        # slab cols AND contiguous bank cols -> one matmul
                        v = vtile(cg, tl)
                        r0 = SLAB_A + max(2 * tl - 1, 0) * P
                        r1 = SLAB_A + min(2 * tl + 1, 8) * P
                        c0 = max(tl - 1, 0) * P
                        nc.tensor.matmul(
                            out=banks[cg][:, c0:c0 + (r1 - r0)],
                            lhsT=v, rhs=csl[:, r0:r1],
                            start=(bank_fresh and tl == 0),
                            stop=(tl == TPG - 1),
                            skip_group_check=True)

                for step, g in enumerate(ORDER_A):
                    sl = build_slab(g)

                    # --- A-stream matmuls: bank g>>2, slice g&3 ---
                    bk = g >> 2
                    first = banks[bk] is None
                    if first:
                        banks[bk] = bpool.tile([P, 512], fp32, tag="bk", name="bk")
                    sli = (g & 3) * P
                    for tl in range(TPG):
                        v = vtile(g, tl)
                        nc.tensor.matmul(
                            out=banks[bk][:, sli:sli + P], lhsT=v,
                            rhs=sl[:, tl * P:(tl + 1) * P],
                            start=(first and tl == 0), stop=False,
                            skip_group_check=True)

                    # --- cons + drain per schedule ---
                    if step in CONS_AT:
                        cg = CONS_AT[step]
                        fresh = banks[cg] is None
                        if fresh:
                            banks[cg] = bpool.tile([P, 512], fp32, tag="bk", name="bk")
                        cons(cg, bank_fresh=fresh)
                    if step in DRAIN_AT:
                        drain(DRAIN_AT[step])

    nc.compile()
    return nc


def _pack_inputs(mem_values, arg_weights, root_filler, op_dist,
                 batch_idx, slot_idx, role_idx):
    """Host-side sharding/packing: index selection and copies only."""
    mem_values = np.ascontiguousarray(mem_values, dtype=np.float32)
    arg_weights = np.asarray(arg_weights, dtype=np.float32)
    root_filler = np.asarray(root_filler, dtype=np.float32)
    op_dist = np.asarray(op_dist, dtype=np.float32)
    batch_idx = np.asarray(batch_idx, dtype=np.int64)
    slot_idx = np.asarray(slot_idx, dtype=np.int64)
    role_idx = np.asarray(role_idx, dtype=np.int64)

    w = arg_weights[batch_idx, slot_idx]  # [N, 4] gathered copies
    r = role_idx
    even = (r & 1) == 0
    wA = np.where(even, w[:, 0], np.where(r != 1, w[:, 1], 0.0))
    opA = np.where(even, op_dist[batch_idx, 0], op_dist[batch_idx, 1])

    iota_np = np.broadcast_to(
        np.arange(P, dtype=np.float16), (P, P)).copy()

    in_maps = []
    for c in range(NCORES):
        vals_s = np.zeros((NT * P, F), np.float16)
        meta_s = np.zeros((BPC, P, 280), np.float16)
        r1pm_s = np.zeros((BPC, P, 40, 2), np.float32)
        r1pm_s[:, :, :, 0] = -1.0
        r1pm_s[:, :, :, 1] = 1.0
        idx_s = np.full((BPC, P, NG, 16), -1, np.int16)

        for bb in range(BPC):
            b = c * BPC + bb
            sel0 = np.nonzero(batch_idx == b)[0]
            rr0 = r[sel0]
            for g in range(NG):
                gsel = sel0[(rr0 >> 8) == g]
                rg = r[gsel]
                if g < 8:
                    j = (rg >> 6) & 3
                    order = np.argsort(j, kind="stable")
                    gsel, rg, j = gsel[order], rg[order], j[order]
                    cnt = np.bincount(j, minlength=4)
                    is_root = np.zeros(rg.size, bool)
                    if g == 0:
                        # synthetic root entry joins block 0's stream end
                        ins = cnt[0]
                        gsel = np.insert(gsel, ins, -1)
                        rg = np.insert(rg, ins, 0)
                        j = np.insert(j, ins, 0)
                        is_root = np.insert(is_root, ins, True)
                        cnt[0] += 1
                    start = np.zeros(4, np.int64)
                    pos_in = np.arange(rg.size) - np.concatenate(
                        [[0], np.cumsum(cnt)])[:-1][j]
                    e = 0
                    for blk in range(4):
                        start[blk] = max(e, 128 * blk)
                        e = start[blk] + cnt[blk]
                    if cnt.max() > 256 or e > SLAB_A or \
                       (start[:3] + cnt[:3] > [256, 384, 512]).any():
                        raise RuntimeError("chain capacity exceeded")
                    pos = start[j] + pos_in
                    tl = pos >> 7
                    if (tl > j + 1).any() or (tl < j).any():
                        raise RuntimeError("chain incidence violated")
                    rng = np.where(tl == j, 2 * j, 2 * j + 1)
                    c2 = SLAB_A + rng * P + 2 * (rg & 63)
                else:
                    order = np.argsort(rg, kind="stable")
                    gsel, rg = gsel[order], rg[order]
                    if rg.size > SLAB_A:
                        raise RuntimeError("upper capacity exceeded")
                    pos = np.arange(rg.size)
                    tl = pos >> 7
                    is_root = np.zeros(rg.size, bool)

                p = pos & 127
                r1 = (rg >> 1) & 127
                acol = tl * P + r1
                slot = g * TPG + tl
                t_global = bb * TILES_PER_BATCH + SPOS[g] * TPG + tl

                real = ~is_root
                vals_s[t_global * P + p] = np.where(
                    is_root[:, None], root_filler[b].astype(np.float16),
                    mem_values[gsel].astype(np.float16))
                # meta1: (WA, OPA) at [slot*2], zeros for root
                meta_s[bb, p[real], slot[real] * 2] = wA[gsel[real]]
                meta_s[bb, p[real], slot[real] * 2 + 1] = opA[gsel[real]]
                # A one-hot idx (col 0..4 by tile) — skip root
                idx_s[bb, p[real], g, tl[real]] = acol[real]
                if g < 8:
                    # meta2: (WB, WC, OP2) at [160 + (g*5+tl)*3]
                    base = 160 + slot * 3
                    meta_s[bb, p[real], base[real]] = w[gsel[real], 2]
                    meta_s[bb, p[real], base[real] + 1] = w[gsel[real], 3]
                    meta_s[bb, p, base + 2] = op_dist[b, 2]
                    if is_root.any():
                        meta_s[bb, p[is_root], base[is_root] + 1] = 1.0
                    idx_s[bb, p[real], g, 5 + tl[real]] = c2[real]
                    idx_s[bb, p, g, 10 + tl] = c2 + 1
                    if is_root.any():
                        # root has no even-bin write
                        idx_s[bb, p[is_root], g, 5 + tl[is_root]] = -1
                else:
                    us = (g - 8) * TPG + tl
                    r1pm_s[bb, p, us, 0] = r1
                    r1pm_s[bb, p, us, 1] = -r1.astype(np.float32)

        in_maps.append({
            "vals": np.ascontiguousarray(
                vals_s.reshape(NVS, VB, P, F).transpose(0, 2, 1, 3)),
            "meta": meta_s,
            "r1pm": r1pm_s,
            "idxs": idx_s,
            "iota": iota_np,
        })
    return in_maps


def kernel(**inputs):
    from concourse.bass_utils import run_bass_kernel_spmd

    in_maps = _pack_inputs(**inputs)
    if "nc" not in _PROG_CACHE:
        _PROG_CACHE["nc"] = _build_program()
    nc = _PROG_CACHE["nc"]
    res = run_bass_kernel_spmd(nc, in_maps, list(range(NCORES)))
    return np.ascontiguousarray(np.concatenate(
        [res.results[c]["out"].transpose(0, 2, 1) for c in range(NCORES)],
        axis=0).astype(np.float32))
